# revision 1
# baseline (speedup 1.0000x reference)
"""GaussianMLP sampling kernel for 8 trn2 NeuronCores (pure data parallel).

reference:
    h      = relu(x @ W_emb + b_emb)        x:[B,128] W_emb:[128,256]
    mean   = h @ W_mean + b_mean            W_mean:[256,128]
    logvar = h @ W_logvar + b_logvar        W_logvar:[256,128]
    z      = mean + exp(0.5*logvar) * eps
    returns (z, mean, logvar)

Sharding: x/eps split along batch across 8 cores; weights replicated.

Per-core dataflow (ROWS_PER_TILE=512 rows/iteration):
  - DMA x tile [128p, 4, 128] (natural layout, partition=row)
  - PE transpose 4x [128,128] -> xT [d_in=128p, 512 rows] in PSUM, copy to SBUF
  - hT = W_emb.T @ x.T via 2 matmuls (lhsT=W_emb chunk, rhs=xT) -> PSUM
  - ACT relu(hT + b_emb) PSUM->SBUF (per-partition bias)
  - mean/logvar: bias seeded by a K=1 ones-matmul over the whole PSUM bank,
    then per-128-row subtile: accumulate hT0.T@Wm0 + hT1.T@Wm1
  - epilogue: ACT copies logvar out + exp(0.5*logvar); DVE mean copy,
    se=std*eps, z=mean+se; DMA 3 outputs
"""

import os
import sys

sys.path.insert(0, "/opt/trn_rl_repo")

import numpy as np

from contextlib import ExitStack

from concourse import bacc, bass, masks, mybir, tile
from concourse.alu_op_type import AluOpType
from concourse.bass_utils import run_bass_kernel_spmd

N_CORES = 8
B = 524288
D_IN = 128
D_H = 256
D_OUT = 128
ROWS_PER_CORE = B // N_CORES  # 65536
ROWS_PER_TILE = 512
N_TILES = ROWS_PER_CORE // ROWS_PER_TILE  # 128
S = ROWS_PER_TILE // 128  # 4 subtiles of 128 rows

F32 = mybir.dt.float32
# dtype for the two matmul layers (float32 = exact-ish, bfloat16 = faster PE)
BF16 = mybir.dt.bfloat16
L1_DT = BF16
L2_DT = BF16


def build_bass(rows_per_core=ROWS_PER_CORE):
    nc = bacc.Bacc("TRN2", target_bir_lowering=False, debug=False)
    n_tiles = rows_per_core // ROWS_PER_TILE

    x_ext = nc.declare_dram_parameter("x", [rows_per_core, D_IN], F32, isOutput=False)
    eps_ext = nc.declare_dram_parameter(
        "eps", [rows_per_core, D_OUT], F32, isOutput=False
    )
    We_ext = nc.declare_dram_parameter("W_emb", [D_IN, D_H], F32, isOutput=False)
    be_ext = nc.declare_dram_parameter("b_emb", [D_H], F32, isOutput=False)
    Wm_ext = nc.declare_dram_parameter("W_mean", [D_H, D_OUT], F32, isOutput=False)
    bm_ext = nc.declare_dram_parameter("b_mean", [D_OUT], F32, isOutput=False)
    Wl_ext = nc.declare_dram_parameter("W_logvar", [D_H, D_OUT], F32, isOutput=False)
    bl_ext = nc.declare_dram_parameter("b_logvar", [D_OUT], F32, isOutput=False)
    z_ext = nc.declare_dram_parameter("z", [rows_per_core, D_OUT], F32, isOutput=True)
    mean_ext = nc.declare_dram_parameter(
        "mean", [rows_per_core, D_OUT], F32, isOutput=True
    )
    lv_ext = nc.declare_dram_parameter(
        "logvar", [rows_per_core, D_OUT], F32, isOutput=True
    )

    # tiled DRAM views: row = t*ROWS_PER_TILE + s*128 + p
    xv = x_ext.rearrange("(t s p) d -> t p s d", s=S, p=128)
    ev = eps_ext.rearrange("(t s p) d -> t p s d", s=S, p=128)
    zv = z_ext.rearrange("(t s p) d -> t p s d", s=S, p=128)
    mv = mean_ext.rearrange("(t s p) d -> t p s d", s=S, p=128)
    lvv = lv_ext.rearrange("(t s p) d -> t p s d", s=S, p=128)

    with tile.TileContext(nc) as tc, ExitStack() as ctx:
        const = ctx.enter_context(tc.tile_pool(name="const", bufs=1))
        xin = ctx.enter_context(tc.tile_pool(name="xin", bufs=3))
        epool = ctx.enter_context(tc.tile_pool(name="eps", bufs=3))
        xTp = ctx.enter_context(tc.tile_pool(name="xT", bufs=3))
        hTp = ctx.enter_context(tc.tile_pool(name="hTs", bufs=2))
        outs = ctx.enter_context(tc.tile_pool(name="outs", bufs=3))
        psA = ctx.enter_context(tc.tile_pool(name="psA", bufs=2, space="PSUM"))
        psB = ctx.enter_context(tc.tile_pool(name="psB", bufs=1, space="PSUM"))
        psC = ctx.enter_context(tc.tile_pool(name="psC", bufs=2, space="PSUM"))

        # --- constants / weights (loaded once) ---
        ident = const.tile([128, 128], F32)
        masks.make_identity(nc, ident[:])

        We_sb = const.tile([128, D_H], L1_DT)
        dma_w = nc.gpsimd if L1_DT != F32 else nc.sync
        dma_w.dma_start(We_sb[:], We_ext[:])

        Wm_sb = const.tile([128, 2, D_OUT], L2_DT)
        Wl_sb = const.tile([128, 2, D_OUT], L2_DT)
        dma_w2 = nc.gpsimd if L2_DT != F32 else nc.sync
        dma_w2.dma_start(Wm_sb[:], Wm_ext.rearrange("(c p) d -> p c d", p=128))
        dma_w2.dma_start(Wl_sb[:], Wl_ext.rearrange("(c p) d -> p c d", p=128))

        be_sb = const.tile([128, 2], F32)
        nc.sync.dma_start(be_sb[:], be_ext.rearrange("(c p) -> p c", p=128))

        ones_sb = const.tile([1, 128], F32)
        nc.vector.memset(ones_sb[:], 1.0)
        bm_rep = const.tile([1, S * D_OUT], F32)
        bl_rep = const.tile([1, S * D_OUT], F32)
        for s in range(S):
            nc.sync.dma_start(
                bm_rep[0:1, s * D_OUT : (s + 1) * D_OUT],
                bm_ext.rearrange("(o d) -> o d", o=1),
            )
            nc.sync.dma_start(
                bl_rep[0:1, s * D_OUT : (s + 1) * D_OUT],
                bl_ext.rearrange("(o d) -> o d", o=1),
            )

        for t in range(n_tiles):
            x_sb = xin.tile([128, S, D_IN], F32, tag="x")
            nc.sync.dma_start(x_sb[:], xv[t])
            eps_sb = epool.tile([128, S, D_OUT], F32, tag="eps")
            nc.sync.dma_start(eps_sb[:], ev[t])

            # transpose x -> xT [d_in, rows]
            xT_ps = psA.tile([128, ROWS_PER_TILE], F32, tag="xT")
            for s in range(S):
                nc.tensor.transpose(
                    xT_ps[:, s * 128 : (s + 1) * 128], x_sb[:, s, :], ident[:]
                )
            xT_sb = xTp.tile([128, ROWS_PER_TILE], L1_DT, tag="xTs")
            nc.vector.tensor_copy(xT_sb[:], xT_ps[:])

            # layer 1: hT[c] = W_emb[:, c].T @ xT  (c: two 128-wide d_h chunks)
            hT_ps0 = psB.tile([128, ROWS_PER_TILE], F32, tag="hT0")
            hT_ps1 = psB.tile([128, ROWS_PER_TILE], F32, tag="hT1")
            nc.tensor.matmul(
                hT_ps0[:], We_sb[:, 0:128], xT_sb[:], start=True, stop=True
            )
            nc.tensor.matmul(
                hT_ps1[:], We_sb[:, 128:256], xT_sb[:], start=True, stop=True
            )
            hT_sb0 = hTp.tile([128, ROWS_PER_TILE], L2_DT, tag="h0")
            hT_sb1 = hTp.tile([128, ROWS_PER_TILE], L2_DT, tag="h1")
            nc.scalar.activation(
                hT_sb0[:],
                hT_ps0[:],
                mybir.ActivationFunctionType.Relu,
                bias=be_sb[:, 0:1],
            )
            nc.scalar.activation(
                hT_sb1[:],
                hT_ps1[:],
                mybir.ActivationFunctionType.Relu,
                bias=be_sb[:, 1:2],
            )

            # layer 2: mean/logvar [rows, d_out] per 128-row subtile,
            # bias seeded across the whole 512-wide bank by a K=1 matmul
            mean_ps = psC.tile([128, S * D_OUT], F32, tag="mean")
            lv_ps = psC.tile([128, S * D_OUT], F32, tag="lv")
            nc.tensor.matmul(
                mean_ps[:], ones_sb[:], bm_rep[:],
                start=True, stop=False, skip_group_check=True,
            )
            nc.tensor.matmul(
                lv_ps[:], ones_sb[:], bl_rep[:],
                start=True, stop=False, skip_group_check=True,
            )
            for s in range(S):
                sl = slice(s * 128, (s + 1) * 128)
                so = slice(s * D_OUT, (s + 1) * D_OUT)
                nc.tensor.matmul(
                    mean_ps[:, so], hT_sb0[:, sl], Wm_sb[:, 0, :],
                    start=False, stop=False, skip_group_check=True,
                )
                nc.tensor.matmul(
                    mean_ps[:, so], hT_sb1[:, sl], Wm_sb[:, 1, :],
                    start=False, stop=(s == S - 1), skip_group_check=True,
                )
                nc.tensor.matmul(
                    lv_ps[:, so], hT_sb0[:, sl], Wl_sb[:, 0, :],
                    start=False, stop=False, skip_group_check=True,
                )
                nc.tensor.matmul(
                    lv_ps[:, so], hT_sb1[:, sl], Wl_sb[:, 1, :],
                    start=False, stop=(s == S - 1), skip_group_check=True,
                )

            # epilogue
            lv_sb = outs.tile([128, S * D_OUT], F32, tag="lvs")
            nc.scalar.activation(
                lv_sb[:], lv_ps[:], mybir.ActivationFunctionType.Copy
            )
            std_sb = outs.tile([128, S * D_OUT], F32, tag="std")
            nc.scalar.activation(
                std_sb[:], lv_ps[:], mybir.ActivationFunctionType.Exp, scale=0.5
            )
            mean_sb = outs.tile([128, S * D_OUT], F32, tag="means")
            nc.vector.tensor_copy(mean_sb[:], mean_ps[:])
            se_sb = outs.tile([128, S * D_OUT], F32, tag="se")
            nc.vector.tensor_mul(
                se_sb[:], std_sb[:], eps_sb[:].rearrange("p s d -> p (s d)")
            )
            z_sb = outs.tile([128, S * D_OUT], F32, tag="z")
            nc.vector.scalar_tensor_tensor(
                z_sb[:], mean_ps[:], 1.0, se_sb[:], AluOpType.mult, AluOpType.add
            )

            nc.sync.dma_start(zv[t], z_sb[:].rearrange("p (s d) -> p s d", s=S))
            nc.sync.dma_start(mv[t], mean_sb[:].rearrange("p (s d) -> p s d", s=S))
            nc.sync.dma_start(lvv[t], lv_sb[:].rearrange("p (s d) -> p s d", s=S))

    nc.finalize()
    return nc


_NC_CACHE = None


def _get_nc():
    global _NC_CACHE
    if _NC_CACHE is None:
        _NC_CACHE = build_bass()
    return _NC_CACHE


def _run(inputs, trace=False, **kw):
    nc = _get_nc()
    xs = np.ascontiguousarray(np.asarray(inputs["x"], dtype=np.float32))
    es = np.ascontiguousarray(np.asarray(inputs["eps"], dtype=np.float32))
    weights = {
        k: np.ascontiguousarray(np.asarray(inputs[k], dtype=np.float32))
        for k in ("W_emb", "b_emb", "W_mean", "b_mean", "W_logvar", "b_logvar")
    }
    in_maps = []
    for c in range(N_CORES):
        sl = slice(c * ROWS_PER_CORE, (c + 1) * ROWS_PER_CORE)
        in_maps.append({"x": xs[sl], "eps": es[sl], **weights})
    res = run_bass_kernel_spmd(nc, in_maps, list(range(N_CORES)), trace=trace, **kw)
    z = np.concatenate([res.results[c]["z"] for c in range(N_CORES)], axis=0)
    mean = np.concatenate([res.results[c]["mean"] for c in range(N_CORES)], axis=0)
    lv = np.concatenate([res.results[c]["logvar"] for c in range(N_CORES)], axis=0)
    return (z, mean, lv), res


def kernel(**inputs):
    out, _ = _run(inputs, trace=False)
    return out


if __name__ == "__main__":
    rng = np.random.default_rng(0)
    demo = {
        "x": rng.standard_normal((B, D_IN), dtype=np.float32),
        "eps": rng.standard_normal((B, D_OUT), dtype=np.float32),
        "W_emb": rng.standard_normal((D_IN, D_H), dtype=np.float32) * 0.088,
        "b_emb": rng.standard_normal((D_H,), dtype=np.float32) * 0.05,
        "W_mean": rng.standard_normal((D_H, D_OUT), dtype=np.float32) * 0.06,
        "b_mean": rng.standard_normal((D_OUT,), dtype=np.float32) * 0.03,
        "W_logvar": rng.standard_normal((D_H, D_OUT), dtype=np.float32) * 0.06,
        "b_logvar": rng.standard_normal((D_OUT,), dtype=np.float32) * 0.03,
    }
    z, m, l = kernel(**demo)
    print("shapes", z.shape, m.shape, l.shape)



# revision 4
# speedup vs baseline: 2.9851x; 2.9851x over previous
"""GaussianMLP sampling kernel for 8 trn2 NeuronCores (pure data parallel).

reference:
    h      = relu(x @ W_emb + b_emb)        x:[B,128] W_emb:[128,256]
    mean   = h @ W_mean + b_mean            W_mean:[256,128]
    logvar = h @ W_logvar + b_logvar        W_logvar:[256,128]
    z      = mean + exp(0.5*logvar) * eps
    returns (z, mean, logvar)

v2 design (transposed dataflow, all-bf16 I/O):
  - Host stages xT/epsT as bf16 [128, rows] per core, weights as bf16,
    so the kernel never transposes on-chip and HBM traffic is halved.
  - All compute keeps the feature dim on partitions:
      hT[dh, r]   = We_chunk.T @ xT          (2 matmuls, K=d_in=128)
      meanT[do,r] = Wm_chunk.T @ hT_chunk    (2 matmuls accum, K=dh)
      lvT [do,r]  = Wl_chunk.T @ hT_chunk    (2 matmuls accum)
    so L2 biases are per-partition -> folded into ACT/DVE/Pool bias ops,
    no PSUM bias seeding and no PE transposes at all.
  - Outputs written bf16 transposed [128, rows]; host casts/transposes back.
  - Epilogue spread across ACT (relu, exp), Pool (relu, mean+bias),
    DVE (lv+bias, std*eps, z-add).
  - DMA in 4096-row super-tiles (1 MiB per stream) on the sync HWDGE queue.
"""

import sys

sys.path.insert(0, "/opt/trn_rl_repo")

import numpy as np
import ml_dtypes

from contextlib import ExitStack

from concourse import bacc, bass, mybir, tile
from concourse.alu_op_type import AluOpType
from concourse.bass_utils import run_bass_kernel_spmd

N_CORES = 8
B = 524288
D_IN = 128
D_H = 256
D_OUT = 128
ROWS_PER_CORE = B // N_CORES  # 65536
R_DMA = 4096  # rows per DMA super-tile
R_SUB = 512  # rows per compute subtile
N_T = ROWS_PER_CORE // R_DMA  # 16
N_U = R_DMA // R_SUB  # 8

F32 = mybir.dt.float32
BF16 = mybir.dt.bfloat16
NP_BF16 = ml_dtypes.bfloat16

AF = mybir.ActivationFunctionType


def build_bass(rows_per_core=ROWS_PER_CORE):
    nc = bacc.Bacc("TRN2", target_bir_lowering=False, debug=False)
    n_t = rows_per_core // R_DMA

    xT_ext = nc.declare_dram_parameter("xT", [D_IN, rows_per_core], BF16, isOutput=False)
    epsT_ext = nc.declare_dram_parameter(
        "epsT", [D_OUT, rows_per_core], BF16, isOutput=False
    )
    We_ext = nc.declare_dram_parameter("W_emb", [D_IN, D_H], BF16, isOutput=False)
    be_ext = nc.declare_dram_parameter("b_emb", [D_H], F32, isOutput=False)
    Wm_ext = nc.declare_dram_parameter("W_mean", [D_H, D_OUT], BF16, isOutput=False)
    bm_ext = nc.declare_dram_parameter("b_mean", [D_OUT], F32, isOutput=False)
    Wl_ext = nc.declare_dram_parameter("W_logvar", [D_H, D_OUT], BF16, isOutput=False)
    bl_ext = nc.declare_dram_parameter("b_logvar", [D_OUT], F32, isOutput=False)
    blh_ext = nc.declare_dram_parameter("b_logvar_half", [D_OUT], F32, isOutput=False)
    zT_ext = nc.declare_dram_parameter("zT", [D_OUT, rows_per_core], BF16, isOutput=True)
    mT_ext = nc.declare_dram_parameter(
        "meanT", [D_OUT, rows_per_core], BF16, isOutput=True
    )
    lT_ext = nc.declare_dram_parameter(
        "logvarT", [D_OUT, rows_per_core], BF16, isOutput=True
    )

    with tile.TileContext(nc) as tc, ExitStack() as ctx:
        const = ctx.enter_context(tc.tile_pool(name="const", bufs=1))
        xin = ctx.enter_context(tc.tile_pool(name="xin", bufs=2))
        epool = ctx.enter_context(tc.tile_pool(name="eps", bufs=2))
        hpool = ctx.enter_context(tc.tile_pool(name="hT", bufs=2))
        spool = ctx.enter_context(tc.tile_pool(name="small", bufs=2))
        stg = ctx.enter_context(tc.tile_pool(name="stg", bufs=2))
        psH = ctx.enter_context(tc.tile_pool(name="psH", bufs=2, space="PSUM"))
        psO = ctx.enter_context(tc.tile_pool(name="psO", bufs=2, space="PSUM"))

        # --- constants / weights (loaded once) ---
        We_sb = const.tile([128, D_H], BF16)
        nc.sync.dma_start(We_sb[:], We_ext[:])
        Wm_sb = const.tile([128, 2, D_OUT], BF16)
        Wl_sb = const.tile([128, 2, D_OUT], BF16)
        nc.sync.dma_start(Wm_sb[:], Wm_ext.rearrange("(c p) d -> p c d", p=128))
        nc.sync.dma_start(Wl_sb[:], Wl_ext.rearrange("(c p) d -> p c d", p=128))
        be_sb = const.tile([128, 2], F32)
        nc.sync.dma_start(be_sb[:], be_ext.rearrange("(c p) -> p c", p=128))
        bm_sb = const.tile([128, 1], F32)
        nc.sync.dma_start(bm_sb[:], bm_ext.rearrange("(p o) -> p o", o=1))
        bl_sb = const.tile([128, 1], F32)
        nc.sync.dma_start(bl_sb[:], bl_ext.rearrange("(p o) -> p o", o=1))
        blh_sb = const.tile([128, 1], F32)
        nc.sync.dma_start(blh_sb[:], blh_ext.rearrange("(p o) -> p o", o=1))

        def emit_l1(t, u, x_sb):
            """L1 for subtile u of DMA tile t: hT psum + relu to SBUF bf16."""
            xs = x_sb[:, u * R_SUB : (u + 1) * R_SUB]
            h_ps0 = psH.tile([128, R_SUB], F32, tag="h0")
            h_ps1 = psH.tile([128, R_SUB], F32, tag="h1")
            nc.tensor.matmul(h_ps0[:], We_sb[:, 0:128], xs, start=True, stop=True)
            nc.tensor.matmul(h_ps1[:], We_sb[:, 128:256], xs, start=True, stop=True)
            h_sb0 = hpool.tile([128, R_SUB], BF16, tag="h0")
            h_sb1 = hpool.tile([128, R_SUB], BF16, tag="h1")
            nc.scalar.activation(h_sb0[:], h_ps0[:], AF.Relu, bias=be_sb[:, 0:1])
            nc.scalar.activation(h_sb1[:], h_ps1[:], AF.Relu, bias=be_sb[:, 1:2])
            return h_sb0, h_sb1

        def emit_l2(t, u, h_sb0, h_sb1, eps_sb, z_st, m_st, l_st):
            """L2 + epilogue for subtile u: writes bf16 slices into staging."""
            sl = slice(u * R_SUB, (u + 1) * R_SUB)
            m_ps = psO.tile([128, R_SUB], F32, tag="m")
            l_ps = psO.tile([128, R_SUB], F32, tag="l")
            nc.tensor.matmul(m_ps[:], Wm_sb[:, 0, :], h_sb0[:], start=True, stop=False)
            nc.tensor.matmul(m_ps[:], Wm_sb[:, 1, :], h_sb1[:], start=False, stop=True)
            nc.tensor.matmul(l_ps[:], Wl_sb[:, 0, :], h_sb0[:], start=True, stop=False)
            nc.tensor.matmul(l_ps[:], Wl_sb[:, 1, :], h_sb1[:], start=False, stop=True)

            # logvar out = lv_ps + bl   (DVE);  std = exp(0.5*lv_ps + 0.5*bl) (ACT)
            nc.vector.tensor_scalar(
                l_st[:, sl], l_ps[:], bl_sb[:, 0:1], None, AluOpType.add
            )
            std_sb = spool.tile([128, R_SUB], BF16, tag="std")
            nc.scalar.activation(std_sb[:], l_ps[:], AF.Exp, bias=blh_sb[:, 0:1], scale=0.5)
            # mean out = m_ps + bm  (DVE)
            nc.vector.tensor_scalar(
                m_st[:, sl], m_ps[:], bm_sb[:, 0:1], None, AluOpType.add
            )
            # z = mean + std*eps  (GpSimd, SBUF-only bf16 ops)
            se_sb = spool.tile([128, R_SUB], BF16, tag="se")
            nc.gpsimd.tensor_tensor(
                se_sb[:], std_sb[:], eps_sb[:, sl], AluOpType.mult
            )
            nc.gpsimd.tensor_tensor(
                z_st[:, sl], m_st[:, sl], se_sb[:], AluOpType.add
            )

        for t in range(n_t):
            c0, c1 = t * R_DMA, (t + 1) * R_DMA
            x_sb = xin.tile([128, R_DMA], BF16, tag="x")
            nc.sync.dma_start(x_sb[:], xT_ext[:, c0:c1])
            eps_sb = epool.tile([128, R_DMA], BF16, tag="e")
            nc.sync.dma_start(eps_sb[:], epsT_ext[:, c0:c1])
            z_st = stg.tile([128, R_DMA], BF16, tag="z")
            m_st = stg.tile([128, R_DMA], BF16, tag="m")
            l_st = stg.tile([128, R_DMA], BF16, tag="l")

            # software-pipelined: emit L1(u) ahead of L2(u-1) so the PE
            # stream never waits on the relu of the subtile it just made
            prev = None
            for u in range(N_U):
                h0, h1 = emit_l1(t, u, x_sb)
                if prev is not None:
                    emit_l2(t, u - 1, *prev, eps_sb, z_st, m_st, l_st)
                prev = (h0, h1)
            emit_l2(t, N_U - 1, *prev, eps_sb, z_st, m_st, l_st)

            nc.sync.dma_start(zT_ext[:, c0:c1], z_st[:])
            nc.sync.dma_start(mT_ext[:, c0:c1], m_st[:])
            nc.sync.dma_start(lT_ext[:, c0:c1], l_st[:])

    nc.finalize()
    return nc


_NC_CACHE = None


def _get_nc():
    global _NC_CACHE
    if _NC_CACHE is None:
        _NC_CACHE = build_bass()
    return _NC_CACHE


def _run(inputs, trace=False, **kw):
    nc = _get_nc()
    f32 = np.float32
    x = np.asarray(inputs["x"], dtype=f32)
    eps = np.asarray(inputs["eps"], dtype=f32)
    weights = {
        "W_emb": np.ascontiguousarray(np.asarray(inputs["W_emb"], f32).astype(NP_BF16)),
        "W_mean": np.ascontiguousarray(np.asarray(inputs["W_mean"], f32).astype(NP_BF16)),
        "W_logvar": np.ascontiguousarray(
            np.asarray(inputs["W_logvar"], f32).astype(NP_BF16)
        ),
        "b_emb": np.ascontiguousarray(np.asarray(inputs["b_emb"], f32)),
        "b_mean": np.ascontiguousarray(np.asarray(inputs["b_mean"], f32)),
        "b_logvar": np.ascontiguousarray(np.asarray(inputs["b_logvar"], f32)),
        "b_logvar_half": np.ascontiguousarray(
            0.5 * np.asarray(inputs["b_logvar"], f32)
        ),
    }
    in_maps = []
    for c in range(N_CORES):
        sl = slice(c * ROWS_PER_CORE, (c + 1) * ROWS_PER_CORE)
        in_maps.append(
            {
                "xT": np.ascontiguousarray(x[sl].T.astype(NP_BF16)),
                "epsT": np.ascontiguousarray(eps[sl].T.astype(NP_BF16)),
                **weights,
            }
        )
    res = run_bass_kernel_spmd(nc, in_maps, list(range(N_CORES)), trace=trace, **kw)

    def gather(name):
        out = np.empty((B, D_OUT), dtype=f32)
        for c in range(N_CORES):
            sl = slice(c * ROWS_PER_CORE, (c + 1) * ROWS_PER_CORE)
            out[sl] = np.asarray(res.results[c][name], dtype=f32).T
        return out

    return (gather("zT"), gather("meanT"), gather("logvarT")), res


def kernel(**inputs):
    out, _ = _run(inputs, trace=False)
    return out


if __name__ == "__main__":
    rng = np.random.default_rng(0)
    demo = {
        "x": rng.standard_normal((B, D_IN), dtype=np.float32),
        "eps": rng.standard_normal((B, D_OUT), dtype=np.float32),
        "W_emb": rng.standard_normal((D_IN, D_H), dtype=np.float32) * 0.088,
        "b_emb": rng.standard_normal((D_H,), dtype=np.float32) * 0.05,
        "W_mean": rng.standard_normal((D_H, D_OUT), dtype=np.float32) * 0.06,
        "b_mean": rng.standard_normal((D_OUT,), dtype=np.float32) * 0.03,
        "W_logvar": rng.standard_normal((D_H, D_OUT), dtype=np.float32) * 0.06,
        "b_logvar": rng.standard_normal((D_OUT,), dtype=np.float32) * 0.03,
    }
    z, m, l = kernel(**demo)
    print("shapes", z.shape, m.shape, l.shape, z.dtype)


# revision 5
# speedup vs baseline: 3.4206x; 1.1459x over previous
"""GaussianMLP sampling kernel for 8 trn2 NeuronCores (pure data parallel).

reference:
    h      = relu(x @ W_emb + b_emb)        x:[B,128] W_emb:[128,256]
    mean   = h @ W_mean + b_mean            W_mean:[256,128]
    logvar = h @ W_logvar + b_logvar        W_logvar:[256,128]
    z      = mean + exp(0.5*logvar) * eps
    returns (z, mean, logvar)

v2 design (transposed dataflow, all-bf16 I/O):
  - Host stages xT/epsT as bf16 [128, rows] per core, weights as bf16,
    so the kernel never transposes on-chip and HBM traffic is halved.
  - All compute keeps the feature dim on partitions:
      hT[dh, r]   = We_chunk.T @ xT          (2 matmuls, K=d_in=128)
      meanT[do,r] = Wm_chunk.T @ hT_chunk    (2 matmuls accum, K=dh)
      lvT [do,r]  = Wl_chunk.T @ hT_chunk    (2 matmuls accum)
    so L2 biases are per-partition -> folded into ACT/DVE/Pool bias ops,
    no PSUM bias seeding and no PE transposes at all.
  - Outputs written bf16 transposed [128, rows]; host casts/transposes back.
  - Epilogue spread across ACT (relu, exp), Pool (relu, mean+bias),
    DVE (lv+bias, std*eps, z-add).
  - DMA in 4096-row super-tiles (1 MiB per stream) on the sync HWDGE queue.
"""

import sys

sys.path.insert(0, "/opt/trn_rl_repo")

import numpy as np
import ml_dtypes

from contextlib import ExitStack

from concourse import bacc, bass, mybir, tile
from concourse.alu_op_type import AluOpType
from concourse.bass_utils import run_bass_kernel_spmd

N_CORES = 8
B = 524288
D_IN = 128
D_H = 256
D_OUT = 128
ROWS_PER_CORE = B // N_CORES  # 65536
R_DMA = 4096  # rows per DMA super-tile
R_SUB = 512  # rows per compute subtile
N_T = ROWS_PER_CORE // R_DMA  # 16
N_U = R_DMA // R_SUB  # 8

F32 = mybir.dt.float32
BF16 = mybir.dt.bfloat16
NP_BF16 = ml_dtypes.bfloat16

AF = mybir.ActivationFunctionType


def build_bass(rows_per_core=ROWS_PER_CORE):
    nc = bacc.Bacc("TRN2", target_bir_lowering=False, debug=False)
    n_t = rows_per_core // R_DMA

    xT_ext = nc.declare_dram_parameter("xT", [D_IN, rows_per_core], BF16, isOutput=False)
    epsT_ext = nc.declare_dram_parameter(
        "epsT", [D_OUT, rows_per_core], BF16, isOutput=False
    )
    We_ext = nc.declare_dram_parameter("W_emb", [D_IN, D_H], BF16, isOutput=False)
    be_ext = nc.declare_dram_parameter("b_emb", [D_H], F32, isOutput=False)
    Wm_ext = nc.declare_dram_parameter("W_mean", [D_H, D_OUT], BF16, isOutput=False)
    bm_ext = nc.declare_dram_parameter("b_mean", [D_OUT], F32, isOutput=False)
    Wl_ext = nc.declare_dram_parameter("W_logvar", [D_H, D_OUT], BF16, isOutput=False)
    bl_ext = nc.declare_dram_parameter("b_logvar", [D_OUT], F32, isOutput=False)
    blh_ext = nc.declare_dram_parameter("b_logvar_half", [D_OUT], F32, isOutput=False)
    zT_ext = nc.declare_dram_parameter("zT", [D_OUT, rows_per_core], BF16, isOutput=True)
    mT_ext = nc.declare_dram_parameter(
        "meanT", [D_OUT, rows_per_core], BF16, isOutput=True
    )
    lT_ext = nc.declare_dram_parameter(
        "logvarT", [D_OUT, rows_per_core], BF16, isOutput=True
    )

    with tile.TileContext(nc) as tc, ExitStack() as ctx:
        const = ctx.enter_context(tc.tile_pool(name="const", bufs=1))
        xin = ctx.enter_context(tc.tile_pool(name="xin", bufs=2))
        epool = ctx.enter_context(tc.tile_pool(name="eps", bufs=2))
        hpool = ctx.enter_context(tc.tile_pool(name="hT", bufs=2))
        spool = ctx.enter_context(tc.tile_pool(name="small", bufs=2))
        stg = ctx.enter_context(tc.tile_pool(name="stg", bufs=2))
        psH = ctx.enter_context(tc.tile_pool(name="psH", bufs=2, space="PSUM"))
        psO = ctx.enter_context(tc.tile_pool(name="psO", bufs=2, space="PSUM"))

        # --- constants / weights (loaded once) ---
        We_sb = const.tile([128, D_H], BF16)
        nc.sync.dma_start(We_sb[:], We_ext[:])
        Wm_sb = const.tile([128, 2, D_OUT], BF16)
        Wl_sb = const.tile([128, 2, D_OUT], BF16)
        nc.sync.dma_start(Wm_sb[:], Wm_ext.rearrange("(c p) d -> p c d", p=128))
        nc.sync.dma_start(Wl_sb[:], Wl_ext.rearrange("(c p) d -> p c d", p=128))
        be_sb = const.tile([128, 2], F32)
        nc.sync.dma_start(be_sb[:], be_ext.rearrange("(c p) -> p c", p=128))
        bm_sb = const.tile([128, 1], F32)
        nc.sync.dma_start(bm_sb[:], bm_ext.rearrange("(p o) -> p o", o=1))
        bl_sb = const.tile([128, 1], F32)
        nc.sync.dma_start(bl_sb[:], bl_ext.rearrange("(p o) -> p o", o=1))
        blh_sb = const.tile([128, 1], F32)
        nc.sync.dma_start(blh_sb[:], blh_ext.rearrange("(p o) -> p o", o=1))

        def emit_l1(t, u, x_sb):
            """L1 for subtile u of DMA tile t: hT psum + relu to SBUF bf16."""
            xs = x_sb[:, u * R_SUB : (u + 1) * R_SUB]
            h_ps0 = psH.tile([128, R_SUB], F32, tag="h0")
            h_ps1 = psH.tile([128, R_SUB], F32, tag="h1")
            nc.tensor.matmul(h_ps0[:], We_sb[:, 0:128], xs, start=True, stop=True)
            nc.tensor.matmul(h_ps1[:], We_sb[:, 128:256], xs, start=True, stop=True)
            h_sb0 = hpool.tile([128, R_SUB], BF16, tag="h0")
            h_sb1 = hpool.tile([128, R_SUB], BF16, tag="h1")
            nc.scalar.activation(h_sb0[:], h_ps0[:], AF.Relu, bias=be_sb[:, 0:1])
            nc.scalar.activation(h_sb1[:], h_ps1[:], AF.Relu, bias=be_sb[:, 1:2])
            return h_sb0, h_sb1

        def emit_l2(t, u, h_sb0, h_sb1, eps_sb, z_st, m_st, l_st):
            """L2 + epilogue for subtile u: writes bf16 slices into staging."""
            sl = slice(u * R_SUB, (u + 1) * R_SUB)
            m_ps = psO.tile([128, R_SUB], F32, tag="m")
            l_ps = psO.tile([128, R_SUB], F32, tag="l")
            nc.tensor.matmul(m_ps[:], Wm_sb[:, 0, :], h_sb0[:], start=True, stop=False)
            nc.tensor.matmul(m_ps[:], Wm_sb[:, 1, :], h_sb1[:], start=False, stop=True)
            nc.tensor.matmul(l_ps[:], Wl_sb[:, 0, :], h_sb0[:], start=True, stop=False)
            nc.tensor.matmul(l_ps[:], Wl_sb[:, 1, :], h_sb1[:], start=False, stop=True)

            # logvar out = lv_ps + bl   (DVE);  std = exp(0.5*lv_ps + 0.5*bl) (ACT)
            nc.vector.tensor_scalar(
                l_st[:, sl], l_ps[:], bl_sb[:, 0:1], None, AluOpType.add
            )
            std_sb = spool.tile([128, R_SUB], BF16, tag="std")
            nc.scalar.activation(std_sb[:], l_ps[:], AF.Exp, bias=blh_sb[:, 0:1], scale=0.5)
            # mean out = m_ps + bm  (DVE)
            nc.vector.tensor_scalar(
                m_st[:, sl], m_ps[:], bm_sb[:, 0:1], None, AluOpType.add
            )
            # z = mean + std*eps  (se on DVE bf16; z-add on GpSimd, SBUF-only)
            se_sb = spool.tile([128, R_SUB], BF16, tag="se")
            nc.vector.tensor_tensor(
                se_sb[:], std_sb[:], eps_sb[:, sl], AluOpType.mult
            )
            nc.gpsimd.tensor_tensor(
                z_st[:, sl], m_st[:, sl], se_sb[:], AluOpType.add
            )

        for t in range(n_t):
            c0, c1 = t * R_DMA, (t + 1) * R_DMA
            x_sb = xin.tile([128, R_DMA], BF16, tag="x")
            nc.sync.dma_start(x_sb[:], xT_ext[:, c0:c1])
            eps_sb = epool.tile([128, R_DMA], BF16, tag="e")
            nc.sync.dma_start(eps_sb[:], epsT_ext[:, c0:c1])
            z_st = stg.tile([128, R_DMA], BF16, tag="z")
            m_st = stg.tile([128, R_DMA], BF16, tag="m")
            l_st = stg.tile([128, R_DMA], BF16, tag="l")

            # software-pipelined: emit L1(u) ahead of L2(u-1) so the PE
            # stream never waits on the relu of the subtile it just made
            prev = None
            for u in range(N_U):
                h0, h1 = emit_l1(t, u, x_sb)
                if prev is not None:
                    emit_l2(t, u - 1, *prev, eps_sb, z_st, m_st, l_st)
                prev = (h0, h1)
            emit_l2(t, N_U - 1, *prev, eps_sb, z_st, m_st, l_st)

            nc.sync.dma_start(zT_ext[:, c0:c1], z_st[:])
            nc.sync.dma_start(mT_ext[:, c0:c1], m_st[:])
            nc.sync.dma_start(lT_ext[:, c0:c1], l_st[:])

    nc.finalize()
    return nc


_NC_CACHE = None


def _get_nc():
    global _NC_CACHE
    if _NC_CACHE is None:
        _NC_CACHE = build_bass()
    return _NC_CACHE


def _run(inputs, trace=False, **kw):
    nc = _get_nc()
    f32 = np.float32
    x = np.asarray(inputs["x"], dtype=f32)
    eps = np.asarray(inputs["eps"], dtype=f32)
    weights = {
        "W_emb": np.ascontiguousarray(np.asarray(inputs["W_emb"], f32).astype(NP_BF16)),
        "W_mean": np.ascontiguousarray(np.asarray(inputs["W_mean"], f32).astype(NP_BF16)),
        "W_logvar": np.ascontiguousarray(
            np.asarray(inputs["W_logvar"], f32).astype(NP_BF16)
        ),
        "b_emb": np.ascontiguousarray(np.asarray(inputs["b_emb"], f32)),
        "b_mean": np.ascontiguousarray(np.asarray(inputs["b_mean"], f32)),
        "b_logvar": np.ascontiguousarray(np.asarray(inputs["b_logvar"], f32)),
        "b_logvar_half": np.ascontiguousarray(
            0.5 * np.asarray(inputs["b_logvar"], f32)
        ),
    }
    in_maps = []
    for c in range(N_CORES):
        sl = slice(c * ROWS_PER_CORE, (c + 1) * ROWS_PER_CORE)
        in_maps.append(
            {
                "xT": np.ascontiguousarray(x[sl].T.astype(NP_BF16)),
                "epsT": np.ascontiguousarray(eps[sl].T.astype(NP_BF16)),
                **weights,
            }
        )
    res = run_bass_kernel_spmd(nc, in_maps, list(range(N_CORES)), trace=trace, **kw)

    def gather(name):
        out = np.empty((B, D_OUT), dtype=f32)
        for c in range(N_CORES):
            sl = slice(c * ROWS_PER_CORE, (c + 1) * ROWS_PER_CORE)
            out[sl] = np.asarray(res.results[c][name], dtype=f32).T
        return out

    return (gather("zT"), gather("meanT"), gather("logvarT")), res


def kernel(**inputs):
    out, _ = _run(inputs, trace=False)
    return out


if __name__ == "__main__":
    rng = np.random.default_rng(0)
    demo = {
        "x": rng.standard_normal((B, D_IN), dtype=np.float32),
        "eps": rng.standard_normal((B, D_OUT), dtype=np.float32),
        "W_emb": rng.standard_normal((D_IN, D_H), dtype=np.float32) * 0.088,
        "b_emb": rng.standard_normal((D_H,), dtype=np.float32) * 0.05,
        "W_mean": rng.standard_normal((D_H, D_OUT), dtype=np.float32) * 0.06,
        "b_mean": rng.standard_normal((D_OUT,), dtype=np.float32) * 0.03,
        "W_logvar": rng.standard_normal((D_H, D_OUT), dtype=np.float32) * 0.06,
        "b_logvar": rng.standard_normal((D_OUT,), dtype=np.float32) * 0.03,
    }
    z, m, l = kernel(**demo)
    print("shapes", z.shape, m.shape, l.shape, z.dtype)
